# revision 13
# baseline (speedup 1.0000x reference)
"""TRN2 Bass kernel for the quantized 4-layer MLP (dense_mlp, 8 cores).

Strategy v2:
  - Data-parallel over batch: each of the 8 cores gets 1024 of 8192 rows.
  - ALL weight quantization on host: weights shipped as integer LEVELS
    (exact in fp16 / fp8e4m3), in p-major contiguous layouts so every
    weight-tile DMA is 128 fat contiguous descriptors.
  - Layer 1: x split into fp16 hi part (16 normal matmuls) + e4m3 lo
    part (8 DoubleRow matmuls): x_lo = e4m3((x - fp16(x)) * 512),
    lo weights = levels/512 (exact in e4m3) so both passes accumulate
    into the same PSUM at natural scale.
  - Layers 2-4: fp8e4 DoubleRow matmuls over integer levels (exact).
  - BN + QuantReLU epilogue fused: ACT per-feature affine, DVE
    round(+C,-C), DVE clip(min 15, max 0) with fp8 output cast.
  - x is DMA'd in kt-chunks spread over 4 queues for a fast start;
    output DMA'd per-tile on its own queue.
"""

import numpy as np
import ml_dtypes

B, D_IN, H, C_OUT = 8192, 2048, 4096, 1000
NCORES = 8
BC = B // NCORES            # 1024 batch rows per core
N4P = 1024                  # padded final output feature dim (1000 -> 1024)
C_ROUND = float(1.5 * 2 ** 23)
EPS = 1e-5
LO_S = 512.0                # residual scale for layer-1 lo pass

_CACHE = {}


def _build_nc():
    import concourse.bass as bass  # noqa: F401
    from concourse import bacc
    import concourse.mybir as mybir
    import concourse.tile as tile

    dt = mybir.dt
    P = 128
    AF = mybir.ActivationFunctionType
    ALU = mybir.AluOpType
    DRS = mybir.MatmulPerfMode.DoubleRowSwInterleave

    nc = bacc.Bacc("TRN2", target_bir_lowering=False)

    KT1 = D_IN // P   # 16
    KT2 = H // P      # 32
    NT_H = H // P     # 32
    NT4 = N4P // P    # 8

    # ---- DRAM I/O (all layouts p-major, per-tile contiguous) ----
    xh_d = [nc.dram_tensor(f"xh{h}", [P, KT1, 512], dt.float16,
                           kind="ExternalInput") for h in range(2)]
    xl_d = [nc.dram_tensor(f"xl{h}", [P, KT1, 512], dt.float8e4,
                           kind="ExternalInput") for h in range(2)]
    w1h_d = nc.dram_tensor("w1h", [P, NT_H, KT1, P], dt.float16,
                           kind="ExternalInput")
    # DoubleRowSwInterleave layouts: per (p, nt, kp) the 2*P entries are
    # the A/B pair columns interleaved in reverse-n order
    w1l_d = nc.dram_tensor("w1l", [P, NT_H, KT1 // 2, 2 * P], dt.float8e4,
                           kind="ExternalInput")
    w2_d = nc.dram_tensor("w2", [P, NT_H, KT2 // 2, 2 * P], dt.float8e4,
                          kind="ExternalInput")
    w3_d = nc.dram_tensor("w3", [P, NT_H, KT2 // 2, 2 * P], dt.float8e4,
                          kind="ExternalInput")
    w4_d = nc.dram_tensor("w4", [P, NT4, KT2 // 2, 2 * P], dt.float8e4,
                          kind="ExternalInput")
    ab1 = nc.dram_tensor("ab1", [H, 2], dt.float32, kind="ExternalInput")
    ab2 = nc.dram_tensor("ab2", [H, 2], dt.float32, kind="ExternalInput")
    ab3 = nc.dram_tensor("ab3", [H, 2], dt.float32, kind="ExternalInput")
    ab4 = nc.dram_tensor("ab4", [N4P, 2], dt.float32, kind="ExternalInput")
    out_t = nc.dram_tensor("out_t", [N4P, BC], dt.float32,
                           kind="ExternalOutput")

    with tile.TileContext(nc) as tc:
        ppool_cm = tc.tile_pool(name="psum", bufs=8, space="PSUM")
        ppool = ppool_cm.__enter__()
        const_cm = tc.tile_pool(name="const", bufs=1)
        cpool = const_cm.__enter__()

        cbias = cpool.tile([P, 1], dt.float32, name="cbias")
        nc.vector.memset(cbias[:], C_ROUND)

        # PE warmup: dummy matmuls run during the input-DMA wait so the
        # HAM clock gate opens before real work arrives (cold MMs are 2x)
        wz = cpool.tile([P, 512], dt.float16, name="wz")
        nc.vector.memset(wz[:], 0.0)
        for _ in range(24):
            wps = ppool.tile([P, 512], dt.float32, name="ps", tag="ps")
            nc.tensor.matmul(wps[:], wz[:, 0:P], wz[:], start=True, stop=True)

        # A1 (layer-1 output levels) - lives through L2
        apool12_cm = tc.tile_pool(name="acts12", bufs=1)
        apool12 = apool12_cm.__enter__()
        A1 = apool12.tile([P, KT2, BC], dt.float8e4, name="A1")

        # shared weight-tile pool for layers 2-4 (cross-layer prefetch)
        qt234_cm = tc.tile_pool(name="qt234", bufs=4)
        qt234 = qt234_cm.__enter__()
        tmp_cm = tc.tile_pool(name="tmpp", bufs=3)
        tmp_pool = tmp_cm.__enter__()
        abt_cm = tc.tile_pool(name="abtp", bufs=1)
        abt_pool = abt_cm.__enter__()

        dma_engs = [nc.sync, nc.gpsimd, nc.scalar]

        def epilogue(psum, abt, nt, b0, a_out, out_stage_pool):
            if a_out is not None:
                tmp = tmp_pool.tile([P, 512], dt.float32, name="tmp", tag="tmp")
                nc.scalar.activation(
                    tmp[:], psum[:], AF.Identity,
                    bias=abt[:, nt, 1:2], scale=abt[:, nt, 0:1])
                nc.vector.tensor_scalar(tmp[:], tmp[:], C_ROUND, C_ROUND,
                                        ALU.add, ALU.subtract)
                nc.vector.tensor_scalar(a_out[:, nt, b0:b0 + 512], tmp[:],
                                        15.0, 0.0, ALU.min, ALU.max)
            else:
                ost = out_stage_pool.tile([P, 512], dt.float32,
                                          name="ost", tag="ost")
                nc.scalar.activation(
                    ost[:], psum[:], AF.Identity,
                    bias=abt[:, nt, 1:2], scale=abt[:, nt, 0:1])
                nc.scalar.dma_start(out_t[nt * P:nt * P + P, b0:b0 + 512],
                                    ost[:])

        def layer234(w_d, ab, NT, a_in, a_out, qt_pool, out_stage_pool=None):
            abt = abt_pool.tile([P, NT, 2], dt.float32, name=f"abt{id(ab)}",
                                tag=f"abt{id(ab)}")
            nc.gpsimd.dma_start(
                abt[:], ab[:].rearrange("(nt p) two -> p nt two", p=P))
            for nt in range(NT):
                qt = qt_pool.tile([P, KT2 // 2, 2 * P], dt.float8e4,
                                  name="qt", tag="qt")
                dma_engs[nt % 2].dma_start(qt[:], w_d[:, nt, :, :])
                for hb in range(2):
                    b0 = hb * 512
                    psum = ppool.tile([P, 512], dt.float32, name="ps", tag="ps")
                    for kp in range(KT2 // 2):
                        nc.tensor.matmul(
                            psum[:], qt[:, kp, :],
                            a_in[:, 2 * kp:2 * kp + 2, b0:b0 + 512],
                            start=(kp == 0), stop=(kp == KT2 // 2 - 1),
                            perf_mode=DRS)
                    epilogue(psum, abt, nt, b0, a_out, out_stage_pool)

        # ---- layer 1 (fp16 hi + e4m3x512 DoubleRow lo) ----
        xt_pool_cm = tc.tile_pool(name="xtp", bufs=1)
        xt_pool = xt_pool_cm.__enter__()
        xh = [xt_pool.tile([P, KT1, 512], dt.float16, name=f"xh{h}")
              for h in range(2)]
        xl = [xt_pool.tile([P, KT1, 512], dt.float8e4, name=f"xl{h}")
              for h in range(2)]

        q1h_cm = tc.tile_pool(name="q1h", bufs=6)
        q1h = q1h_cm.__enter__()
        q1l_cm = tc.tile_pool(name="q1l", bufs=6)
        q1l = q1l_cm.__enter__()

        abt1 = abt_pool.tile([P, NT_H, 2], dt.float32, name="abt1", tag="abt1")
        nc.scalar.dma_start(
            abt1[:], ab1[:].rearrange("(nt p) two -> p nt two", p=P))

        qh_t, ql_t = {}, {}

        def l1_wdma(nt):
            qh_t[nt] = q1h.tile([P, KT1, P], dt.float16, name="qh", tag="qh")
            nc.sync.dma_start(qh_t[nt][:], w1h_d[:, nt, :, :])
            ql_t[nt] = q1l.tile([P, KT1 // 2, 2 * P], dt.float8e4,
                                name="ql", tag="ql")
            nc.gpsimd.dma_start(ql_t[nt][:], w1l_d[:, nt, :, :])

        # DMA priority: nt=0 weights, x half 0, nt=1-3 weights, x half 1.
        # Chain order below consumes h=0 for nt=0..3 first, giving half 1
        # time to arrive before its first chain.
        l1_wdma(0)
        qi = 0
        for c0 in range(0, KT1, 4):
            dma_engs[qi % 3].dma_start(xh[0][:, c0:c0 + 4, :],
                                       xh_d[0][:, c0:c0 + 4, :])
            qi += 1
            dma_engs[qi % 3].dma_start(xl[0][:, c0:c0 + 4, :],
                                       xl_d[0][:, c0:c0 + 4, :])
            qi += 1
        for nt in range(1, 4):
            l1_wdma(nt)
        for c0 in range(0, KT1, 4):
            dma_engs[qi % 3].dma_start(xh[1][:, c0:c0 + 4, :],
                                       xh_d[1][:, c0:c0 + 4, :])
            qi += 1
            dma_engs[qi % 3].dma_start(xl[1][:, c0:c0 + 4, :],
                                       xl_d[1][:, c0:c0 + 4, :])
            qi += 1

        LOOK = 4
        order = [(nt, 0) for nt in range(LOOK)] + \
                [(nt, 1) for nt in range(LOOK)] + \
                [(nt, hb) for nt in range(LOOK, NT_H) for hb in range(2)]
        for nt, hb in order:
            if nt not in qh_t:
                l1_wdma(nt)
            qh, ql = qh_t[nt], ql_t[nt]
            psum = ppool.tile([P, 512], dt.float32, name="ps", tag="ps")
            for kt in range(KT1):
                nc.tensor.matmul(
                    psum[:], qh[:, kt, :], xh[hb][:, kt, :],
                    start=(kt == 0), stop=False)
            for kp in range(KT1 // 2):
                nc.tensor.matmul(
                    psum[:], ql[:, kp, :],
                    xl[hb][:, 2 * kp:2 * kp + 2, :],
                    start=False, stop=(kp == KT1 // 2 - 1),
                    perf_mode=DRS)
            epilogue(psum, abt1, nt, hb * 512, A1, None)

        q1l_cm.__exit__(None, None, None)
        q1h_cm.__exit__(None, None, None)
        xt_pool_cm.__exit__(None, None, None)

        # ---- layers 2-4 (fp8 DoubleRow) ----
        apool23_cm = tc.tile_pool(name="acts23", bufs=1)
        apool23 = apool23_cm.__enter__()
        A2 = apool23.tile([P, KT2, BC], dt.float8e4, name="A2")
        layer234(w2_d, ab2, NT_H, A1, A2, qt234)

        A3 = apool23.tile([P, KT2, BC], dt.float8e4, name="A3")
        layer234(w3_d, ab3, NT_H, A2, A3, qt234)

        ost_cm = tc.tile_pool(name="ostp", bufs=3)
        ost_pool = ost_cm.__enter__()
        layer234(w4_d, ab4, NT4, A3, None, qt234, out_stage_pool=ost_pool)

        ost_cm.__exit__(None, None, None)
        apool23_cm.__exit__(None, None, None)
        abt_cm.__exit__(None, None, None)
        tmp_cm.__exit__(None, None, None)
        qt234_cm.__exit__(None, None, None)
        apool12_cm.__exit__(None, None, None)
        const_cm.__exit__(None, None, None)
        ppool_cm.__exit__(None, None, None)

    nc.compile()
    return nc


def _host_prep(inputs):
    f32 = np.float32
    f8 = ml_dtypes.float8_e4m3
    P = 128

    def wlevels(W):
        # mimic reference: s = max(|W|) / 3.0, levels = round(W/s) in fp32
        s = f32(np.max(np.abs(W))) / f32(3.0)
        lv = np.clip(np.round(W / s), -3.0, 3.0).astype(f32)
        return lv, s

    W1l, sw1 = wlevels(inputs["W1"])
    W2l, sw2 = wlevels(inputs["W2"])
    W3l, sw3 = wlevels(inputs["W3"])
    W4l, sw4 = wlevels(inputs["W4"])
    s_w = [sw1, sw2, sw3, sw4]
    s_a = [f32(inputs[k][0]) for k in ("s1", "s2", "s3")]

    def fold(l, s_prev):
        g = inputs[f"g{l}"].astype(np.float64)
        be = inputs[f"be{l}"].astype(np.float64)
        m = inputs[f"m{l}"].astype(np.float64)
        v = inputs[f"v{l}"].astype(np.float64)
        b = inputs[f"b{l}"].astype(np.float64)
        inv = 1.0 / np.sqrt(v + EPS)
        sl = float(s_a[l - 1])
        alpha = (float(s_prev) * float(s_w[l - 1]) * g * inv) / sl
        beta = ((b - m) * inv * g + be) / sl
        return alpha.astype(f32), beta.astype(f32)

    a1, b1 = fold(1, 1.0)
    a2, b2 = fold(2, s_a[0])
    a3, b3 = fold(3, s_a[1])
    a4 = np.full(N4P, float(s_a[2]) * float(s_w[3]), dtype=f32)
    b4 = np.zeros(N4P, dtype=f32)
    b4[:C_OUT] = inputs["b4"]

    def abpack(a, b):
        return np.ascontiguousarray(np.stack([a, b], axis=1))

    def wtile(Wl, dtype, scale=1.0):
        # [N, K] levels -> [P, NT, KT, P] with lhsT[k, n] = W[n, k]
        N, K = Wl.shape
        NT, KT = N // P, K // P
        arr = (Wl.T * f32(scale)).reshape(KT, P, NT, P)   # [kt, p, nt, nw]
        arr = arr.transpose(1, 2, 0, 3)                   # [p, nt, kt, nw]
        return np.ascontiguousarray(arr).astype(dtype)

    def swi(arr4):
        # [P, NT, KT, n] -> DoubleRowSwInterleave layout [P, NT, KT//2, 2n]:
        # out[p, nt, kp, 2m + i] = arr4[p, nt, 2kp + i, n-1-m]
        Pd, NT, KT, N = arr4.shape
        a = arr4.reshape(Pd, NT, KT // 2, 2, N)[..., ::-1]
        return np.ascontiguousarray(
            a.transpose(0, 1, 2, 4, 3).reshape(Pd, NT, KT // 2, 2 * N))

    w1h = wtile(W1l, np.float16)
    w1l = swi(wtile(W1l, f8, scale=1.0 / LO_S))
    w2 = swi(wtile(W2l, f8))
    w3 = swi(wtile(W3l, f8))
    W4p = np.zeros((N4P, H), dtype=f32)
    W4p[:C_OUT] = W4l
    w4 = swi(wtile(W4p, f8))

    shared = dict(
        w1h=w1h, w1l=w1l, w2=w2, w3=w3, w4=w4,
        ab1=abpack(a1, b1), ab2=abpack(a2, b2), ab3=abpack(a3, b3),
        ab4=abpack(a4, b4),
    )

    KT1 = D_IN // P
    xt = inputs["x"].T  # [D_IN, B] view
    in_maps = []
    for c in range(NCORES):
        xs = np.ascontiguousarray(xt[:, c * BC:(c + 1) * BC], dtype=f32)
        xhi = xs.astype(np.float16)
        xlo = ((xs - xhi.astype(f32)) * f32(LO_S)).astype(f8)
        # [K, B] -> [P, KT, B] p-major, then split batch halves
        xhi_t = xhi.reshape(KT1, P, BC).transpose(1, 0, 2)
        xlo_t = xlo.reshape(KT1, P, BC).transpose(1, 0, 2)
        m = dict(shared)
        for h in range(2):
            m[f"xh{h}"] = np.ascontiguousarray(xhi_t[:, :, h * 512:(h + 1) * 512])
            m[f"xl{h}"] = np.ascontiguousarray(xlo_t[:, :, h * 512:(h + 1) * 512])
        in_maps.append(m)
    return in_maps


def kernel(**inputs):
    from concourse.bass_utils import run_bass_kernel_spmd

    inputs = {k: np.asarray(v) for k, v in inputs.items()}
    if "nc" not in _CACHE:
        _CACHE["nc"] = _build_nc()
    nc = _CACHE["nc"]

    in_maps = _host_prep(inputs)
    res = run_bass_kernel_spmd(nc, in_maps, core_ids=list(range(NCORES)))

    out = np.empty((B, C_OUT), dtype=np.float32)
    for c in range(NCORES):
        out[c * BC:(c + 1) * BC, :] = res.results[c]["out_t"][:C_OUT, :].T
    return out


# revision 15
# speedup vs baseline: 1.0197x; 1.0197x over previous
"""TRN2 Bass kernel for the quantized 4-layer MLP (dense_mlp, 8 cores).

Strategy v2:
  - Data-parallel over batch: each of the 8 cores gets 1024 of 8192 rows.
  - ALL weight quantization on host: weights shipped as integer LEVELS
    (exact in fp16 / fp8e4m3), in p-major contiguous layouts so every
    weight-tile DMA is 128 fat contiguous descriptors.
  - Layer 1: x split into fp16 hi part (16 normal matmuls) + e4m3 lo
    part (8 DoubleRow matmuls): x_lo = e4m3((x - fp16(x)) * 512),
    lo weights = levels/512 (exact in e4m3) so both passes accumulate
    into the same PSUM at natural scale.
  - Layers 2-4: fp8e4 DoubleRow matmuls over integer levels (exact).
  - BN + QuantReLU epilogue fused: ACT per-feature affine, DVE
    round(+C,-C), DVE clip(min 15, max 0) with fp8 output cast.
  - x is DMA'd in kt-chunks spread over 4 queues for a fast start;
    output DMA'd per-tile on its own queue.
"""

import numpy as np
import ml_dtypes

B, D_IN, H, C_OUT = 8192, 2048, 4096, 1000
NCORES = 8
BC = B // NCORES            # 1024 batch rows per core
N4P = 1024                  # padded final output feature dim (1000 -> 1024)
C_ROUND = float(1.5 * 2 ** 23)
EPS = 1e-5
LO_S = 512.0                # residual scale for layer-1 lo pass

_CACHE = {}


def _build_nc():
    import concourse.bass as bass  # noqa: F401
    from concourse import bacc
    import concourse.mybir as mybir
    import concourse.tile as tile

    dt = mybir.dt
    P = 128
    AF = mybir.ActivationFunctionType
    ALU = mybir.AluOpType
    DRS = mybir.MatmulPerfMode.DoubleRowSwInterleave

    nc = bacc.Bacc("TRN2", target_bir_lowering=False)

    KT1 = D_IN // P   # 16
    KT2 = H // P      # 32
    NT_H = H // P     # 32
    NT4 = N4P // P    # 8

    # ---- DRAM I/O (all layouts p-major, per-tile contiguous) ----
    xh_d = [nc.dram_tensor(f"xh{h}", [P, KT1, 512], dt.float16,
                           kind="ExternalInput") for h in range(2)]
    xl_d = [nc.dram_tensor(f"xl{h}", [P, KT1, 512], dt.float8e4,
                           kind="ExternalInput") for h in range(2)]
    w1h_d = nc.dram_tensor("w1h", [P, NT_H, KT1, P], dt.float16,
                           kind="ExternalInput")
    # DoubleRowSwInterleave layouts: per (p, nt, kp) the 2*P entries are
    # the A/B pair columns interleaved in reverse-n order
    w1l_d = nc.dram_tensor("w1l", [P, NT_H, KT1 // 2, 2 * P], dt.float8e4,
                           kind="ExternalInput")
    w2_d = nc.dram_tensor("w2", [P, NT_H, KT2 // 2, 2 * P], dt.float8e4,
                          kind="ExternalInput")
    w3_d = nc.dram_tensor("w3", [P, NT_H, KT2 // 2, 2 * P], dt.float8e4,
                          kind="ExternalInput")
    w4_d = nc.dram_tensor("w4", [P, NT4, KT2 // 2, 2 * P], dt.float8e4,
                          kind="ExternalInput")
    ab1 = nc.dram_tensor("ab1", [H, 2], dt.float32, kind="ExternalInput")
    ab2 = nc.dram_tensor("ab2", [H, 2], dt.float32, kind="ExternalInput")
    ab3 = nc.dram_tensor("ab3", [H, 2], dt.float32, kind="ExternalInput")
    ab4 = nc.dram_tensor("ab4", [N4P, 2], dt.float32, kind="ExternalInput")
    out_t = nc.dram_tensor("out_t", [N4P, BC], dt.float32,
                           kind="ExternalOutput")

    with tile.TileContext(nc) as tc:
        ppool_cm = tc.tile_pool(name="psum", bufs=8, space="PSUM")
        ppool = ppool_cm.__enter__()
        const_cm = tc.tile_pool(name="const", bufs=1)
        cpool = const_cm.__enter__()

        cbias = cpool.tile([P, 1], dt.float32, name="cbias")
        nc.vector.memset(cbias[:], C_ROUND)

        # PE warmup: dummy matmuls run during the input-DMA wait so the
        # HAM clock gate opens before real work arrives (cold MMs are 2x)
        wz = cpool.tile([P, 512], dt.float16, name="wz")
        nc.vector.memset(wz[:], 0.0)
        for _ in range(16):
            wps = ppool.tile([P, 512], dt.float32, name="ps", tag="ps")
            nc.tensor.matmul(wps[:], wz[:, 0:P], wz[:], start=True, stop=True)

        # A1 (layer-1 output levels) - lives through L2
        apool12_cm = tc.tile_pool(name="acts12", bufs=1)
        apool12 = apool12_cm.__enter__()
        A1 = apool12.tile([P, KT2, BC], dt.float8e4, name="A1")

        # shared weight-tile pool for layers 2-4 (cross-layer prefetch)
        qt234_cm = tc.tile_pool(name="qt234", bufs=4)
        qt234 = qt234_cm.__enter__()
        tmp_cm = tc.tile_pool(name="tmpp", bufs=3)
        tmp_pool = tmp_cm.__enter__()
        abt_cm = tc.tile_pool(name="abtp", bufs=1)
        abt_pool = abt_cm.__enter__()

        dma_engs = [nc.sync, nc.gpsimd, nc.scalar]

        def epilogue(psum, abt, nt, b0, a_out, out_stage_pool):
            if a_out is not None:
                tmp = tmp_pool.tile([P, 512], dt.float32, name="tmp", tag="tmp")
                nc.scalar.activation(
                    tmp[:], psum[:], AF.Identity,
                    bias=abt[:, nt, 1:2], scale=abt[:, nt, 0:1])
                nc.vector.tensor_scalar(tmp[:], tmp[:], C_ROUND, C_ROUND,
                                        ALU.add, ALU.subtract)
                nc.vector.tensor_scalar(a_out[:, nt, b0:b0 + 512], tmp[:],
                                        15.0, 0.0, ALU.min, ALU.max)
            else:
                ost = out_stage_pool.tile([P, 512], dt.float32,
                                          name="ost", tag="ost")
                nc.scalar.activation(
                    ost[:], psum[:], AF.Identity,
                    bias=abt[:, nt, 1:2], scale=abt[:, nt, 0:1])
                nc.scalar.dma_start(out_t[nt * P:nt * P + P, b0:b0 + 512],
                                    ost[:])

        def layer234(w_d, ab, NT, a_in, a_out, qt_pool, out_stage_pool=None):
            abt = abt_pool.tile([P, NT, 2], dt.float32, name=f"abt{id(ab)}",
                                tag=f"abt{id(ab)}")
            nc.gpsimd.dma_start(
                abt[:], ab[:].rearrange("(nt p) two -> p nt two", p=P))
            for nt in range(NT):
                qt = qt_pool.tile([P, KT2 // 2, 2 * P], dt.float8e4,
                                  name="qt", tag="qt")
                dma_engs[nt % 2].dma_start(qt[:], w_d[:, nt, :, :])
                for hb in range(2):
                    b0 = hb * 512
                    psum = ppool.tile([P, 512], dt.float32, name="ps", tag="ps")
                    for kp in range(KT2 // 2):
                        nc.tensor.matmul(
                            psum[:], qt[:, kp, :],
                            a_in[:, 2 * kp:2 * kp + 2, b0:b0 + 512],
                            start=(kp == 0), stop=(kp == KT2 // 2 - 1),
                            perf_mode=DRS)
                    epilogue(psum, abt, nt, b0, a_out, out_stage_pool)

        # ---- layer 1 (fp16 hi + e4m3x512 DoubleRow lo) ----
        xt_pool_cm = tc.tile_pool(name="xtp", bufs=1)
        xt_pool = xt_pool_cm.__enter__()
        xh = [xt_pool.tile([P, KT1, 512], dt.float16, name=f"xh{h}")
              for h in range(2)]
        xl = [xt_pool.tile([P, KT1, 512], dt.float8e4, name=f"xl{h}")
              for h in range(2)]

        q1h_cm = tc.tile_pool(name="q1h", bufs=6)
        q1h = q1h_cm.__enter__()
        q1l_cm = tc.tile_pool(name="q1l", bufs=6)
        q1l = q1l_cm.__enter__()

        abt1 = abt_pool.tile([P, NT_H, 2], dt.float32, name="abt1", tag="abt1")
        nc.scalar.dma_start(
            abt1[:], ab1[:].rearrange("(nt p) two -> p nt two", p=P))

        qh_t, ql_t = {}, {}

        def l1_wdma(nt):
            qh_t[nt] = q1h.tile([P, KT1, P], dt.float16, name="qh", tag="qh")
            nc.sync.dma_start(qh_t[nt][:], w1h_d[:, nt, :, :])
            ql_t[nt] = q1l.tile([P, KT1 // 2, 2 * P], dt.float8e4,
                                name="ql", tag="ql")
            nc.gpsimd.dma_start(ql_t[nt][:], w1l_d[:, nt, :, :])

        # DMA order (few fat transfers; hi-x/fp16-w on sync, lo-x/fp8-w on
        # gpsimd; scalar stays free for epilogues): nt=0 weights, x half 0
        # in two chunks, nt=1-3 weights, x half 1. Chain order below
        # consumes h=0 for nt=0..3 first so half 1 has time to arrive.
        l1_wdma(0)
        nc.sync.dma_start(xh[0][:, 0:8, :], xh_d[0][:, 0:8, :])
        nc.gpsimd.dma_start(xl[0][:, 0:8, :], xl_d[0][:, 0:8, :])
        nc.sync.dma_start(xh[0][:, 8:16, :], xh_d[0][:, 8:16, :])
        nc.gpsimd.dma_start(xl[0][:, 8:16, :], xl_d[0][:, 8:16, :])
        for nt in range(1, 4):
            l1_wdma(nt)
        nc.sync.dma_start(xh[1][:], xh_d[1][:])
        nc.gpsimd.dma_start(xl[1][:], xl_d[1][:])

        LOOK = 4
        order = [(nt, 0) for nt in range(LOOK)] + \
                [(nt, 1) for nt in range(LOOK)] + \
                [(nt, hb) for nt in range(LOOK, NT_H) for hb in range(2)]
        for nt, hb in order:
            if nt not in qh_t:
                l1_wdma(nt)
            qh, ql = qh_t[nt], ql_t[nt]
            psum = ppool.tile([P, 512], dt.float32, name="ps", tag="ps")
            for kt in range(KT1):
                nc.tensor.matmul(
                    psum[:], qh[:, kt, :], xh[hb][:, kt, :],
                    start=(kt == 0), stop=False)
            for kp in range(KT1 // 2):
                nc.tensor.matmul(
                    psum[:], ql[:, kp, :],
                    xl[hb][:, 2 * kp:2 * kp + 2, :],
                    start=False, stop=(kp == KT1 // 2 - 1),
                    perf_mode=DRS)
            epilogue(psum, abt1, nt, hb * 512, A1, None)

        q1l_cm.__exit__(None, None, None)
        q1h_cm.__exit__(None, None, None)
        xt_pool_cm.__exit__(None, None, None)

        # ---- layers 2-4 (fp8 DoubleRow) ----
        apool23_cm = tc.tile_pool(name="acts23", bufs=1)
        apool23 = apool23_cm.__enter__()
        A2 = apool23.tile([P, KT2, BC], dt.float8e4, name="A2")
        layer234(w2_d, ab2, NT_H, A1, A2, qt234)

        A3 = apool23.tile([P, KT2, BC], dt.float8e4, name="A3")
        layer234(w3_d, ab3, NT_H, A2, A3, qt234)

        ost_cm = tc.tile_pool(name="ostp", bufs=3)
        ost_pool = ost_cm.__enter__()
        layer234(w4_d, ab4, NT4, A3, None, qt234, out_stage_pool=ost_pool)

        ost_cm.__exit__(None, None, None)
        apool23_cm.__exit__(None, None, None)
        abt_cm.__exit__(None, None, None)
        tmp_cm.__exit__(None, None, None)
        qt234_cm.__exit__(None, None, None)
        apool12_cm.__exit__(None, None, None)
        const_cm.__exit__(None, None, None)
        ppool_cm.__exit__(None, None, None)

    nc.compile()
    return nc


def _host_prep(inputs):
    f32 = np.float32
    f8 = ml_dtypes.float8_e4m3
    P = 128

    def wlevels(W):
        # mimic reference: s = max(|W|) / 3.0, levels = round(W/s) in fp32
        s = f32(np.max(np.abs(W))) / f32(3.0)
        lv = np.clip(np.round(W / s), -3.0, 3.0).astype(f32)
        return lv, s

    W1l, sw1 = wlevels(inputs["W1"])
    W2l, sw2 = wlevels(inputs["W2"])
    W3l, sw3 = wlevels(inputs["W3"])
    W4l, sw4 = wlevels(inputs["W4"])
    s_w = [sw1, sw2, sw3, sw4]
    s_a = [f32(inputs[k][0]) for k in ("s1", "s2", "s3")]

    def fold(l, s_prev):
        g = inputs[f"g{l}"].astype(np.float64)
        be = inputs[f"be{l}"].astype(np.float64)
        m = inputs[f"m{l}"].astype(np.float64)
        v = inputs[f"v{l}"].astype(np.float64)
        b = inputs[f"b{l}"].astype(np.float64)
        inv = 1.0 / np.sqrt(v + EPS)
        sl = float(s_a[l - 1])
        alpha = (float(s_prev) * float(s_w[l - 1]) * g * inv) / sl
        beta = ((b - m) * inv * g + be) / sl
        return alpha.astype(f32), beta.astype(f32)

    a1, b1 = fold(1, 1.0)
    a2, b2 = fold(2, s_a[0])
    a3, b3 = fold(3, s_a[1])
    a4 = np.full(N4P, float(s_a[2]) * float(s_w[3]), dtype=f32)
    b4 = np.zeros(N4P, dtype=f32)
    b4[:C_OUT] = inputs["b4"]

    def abpack(a, b):
        return np.ascontiguousarray(np.stack([a, b], axis=1))

    def wtile(Wl, dtype, scale=1.0):
        # [N, K] levels -> [P, NT, KT, P] with lhsT[k, n] = W[n, k]
        N, K = Wl.shape
        NT, KT = N // P, K // P
        arr = (Wl.T * f32(scale)).reshape(KT, P, NT, P)   # [kt, p, nt, nw]
        arr = arr.transpose(1, 2, 0, 3)                   # [p, nt, kt, nw]
        return np.ascontiguousarray(arr).astype(dtype)

    def swi(arr4):
        # [P, NT, KT, n] -> DoubleRowSwInterleave layout [P, NT, KT//2, 2n]:
        # out[p, nt, kp, 2m + i] = arr4[p, nt, 2kp + i, n-1-m]
        Pd, NT, KT, N = arr4.shape
        a = arr4.reshape(Pd, NT, KT // 2, 2, N)[..., ::-1]
        return np.ascontiguousarray(
            a.transpose(0, 1, 2, 4, 3).reshape(Pd, NT, KT // 2, 2 * N))

    w1h = wtile(W1l, np.float16)
    w1l = swi(wtile(W1l, f8, scale=1.0 / LO_S))
    w2 = swi(wtile(W2l, f8))
    w3 = swi(wtile(W3l, f8))
    W4p = np.zeros((N4P, H), dtype=f32)
    W4p[:C_OUT] = W4l
    w4 = swi(wtile(W4p, f8))

    shared = dict(
        w1h=w1h, w1l=w1l, w2=w2, w3=w3, w4=w4,
        ab1=abpack(a1, b1), ab2=abpack(a2, b2), ab3=abpack(a3, b3),
        ab4=abpack(a4, b4),
    )

    KT1 = D_IN // P
    xt = inputs["x"].T  # [D_IN, B] view
    in_maps = []
    for c in range(NCORES):
        xs = np.ascontiguousarray(xt[:, c * BC:(c + 1) * BC], dtype=f32)
        xhi = xs.astype(np.float16)
        xlo = ((xs - xhi.astype(f32)) * f32(LO_S)).astype(f8)
        # [K, B] -> [P, KT, B] p-major, then split batch halves
        xhi_t = xhi.reshape(KT1, P, BC).transpose(1, 0, 2)
        xlo_t = xlo.reshape(KT1, P, BC).transpose(1, 0, 2)
        m = dict(shared)
        for h in range(2):
            m[f"xh{h}"] = np.ascontiguousarray(xhi_t[:, :, h * 512:(h + 1) * 512])
            m[f"xl{h}"] = np.ascontiguousarray(xlo_t[:, :, h * 512:(h + 1) * 512])
        in_maps.append(m)
    return in_maps


def kernel(**inputs):
    from concourse.bass_utils import run_bass_kernel_spmd

    inputs = {k: np.asarray(v) for k, v in inputs.items()}
    if "nc" not in _CACHE:
        _CACHE["nc"] = _build_nc()
    nc = _CACHE["nc"]

    in_maps = _host_prep(inputs)
    res = run_bass_kernel_spmd(nc, in_maps, core_ids=list(range(NCORES)))

    out = np.empty((B, C_OUT), dtype=np.float32)
    for c in range(NCORES):
        out[c * BC:(c + 1) * BC, :] = res.results[c]["out_t"][:C_OUT, :].T
    return out
